# revision 1
# baseline (speedup 1.0000x reference)
"""Trainium2 Bass kernel for nn_Attention_71966472012100.

Multi-head attention layer (dense_transformer), B=4, S=2048, H=12, D=100,
HID=1200, with the reference's bug-faithful head-mixing reshape before the
output projection.

Sharding: 8 cores = data-parallel over batch (4) x tensor-parallel over head
groups (2 groups of 6 heads). Because the reference reshapes (b, h, s, d) ->
(b, s, h*d) WITHOUT permuting heads back, head-group g's attention output
occupies exactly rows [g*1024, (g+1)*1024) of the reshaped activation
(6*2048*100 = 1024*1200). Each core therefore produces 1024 complete rows of
the final output and no cross-core communication is needed.

Everything on-chip is kept "transposed" so no transposes are ever needed:
  QT/KT: [d, s] per head   (proj matmuls with x^T as moving operand)
  scores^T: [t, s]         (softmax axis t on partitions; alibi+mask become a
                            per-partition bias fused into the ACT exp)
  V: [t, d | ones]         (ones column makes PV also produce softmax sums)
  O^T: [d, h*S+s]          (strided column views of O^T are exactly the lhsT
                            tiles of the bug-faithful output projection)

Matmuls run in float32r (FP22 multiply, FP32 accumulate) which streams at
full PE rate for moving dims >= 256.

Phase order per core (one x^T pass, no re-streaming):
  P1: x^T chunks stream in interleaved with V' accumulation; Q/K projections
      for BOTH head groups while x^T is resident (group 1 spilled to DRAM).
  A0: attention for group 0; group 1's Q/K read back during/after.
  A1: attention for group 1. Per-head softmax normalization is emitted inline
      so its PE work fills the ACT-bound bubbles.
  WO: output projection from strided O^T views.
"""

import numpy as np
from contextlib import ExitStack

import concourse.bass as bass
import concourse.tile as tile
from concourse import bacc
from concourse import mybir
from concourse.bass_utils import run_bass_kernel_spmd

F32 = mybir.dt.float32
F32R = mybir.dt.float32r
EXP = mybir.ActivationFunctionType.Exp

B, S, H, D, HID = 4, 2048, 12, 100, 1200
HG = 2                # head groups (tensor parallel)
HL = H // HG          # 6 heads per core
ROWS = S * HL * D // HID   # 1024 output rows per core
CK, CCH = 120, 10     # contraction chunking of HID
TT = S // 128         # 16 key tiles
VW = HL * D + HL      # 606: V' row width per t-tile (d cols + ones col per head)


def _mm(nc, out, lhsT, rhs, **kw):
    nc.tensor.matmul(out, lhsT.bitcast(F32R), rhs.bitcast(F32R), **kw)


def _absorb(nc, ap):
    """PE-side observation of a freshly DMA'd tile.

    fp32r matmuls lower to an LDW+MM pair with limited sync-wait slots; a
    1-column throwaway bf16 LDWEIGHTS absorbs a DMA wait ahead of time
    (weights are overwritten by the next self-loading fp32r matmul).
    """
    bb = ap.bitcast(mybir.dt.bfloat16)
    nc.tensor.ldweights(bb[:, 0:1])


def build_program(scale: float, debug_taps: bool = False, n_iters: int = 1):
    nc = bacc.Bacc("TRN2", target_bir_lowering=False, debug=False)

    tn = {}
    tn["xT"] = nc.dram_tensor("xT", [HID, S], F32R, kind="ExternalInput")
    tn["wqT"] = nc.dram_tensor("wqT", [HID, HL * D], F32R, kind="ExternalInput")
    tn["wkT"] = nc.dram_tensor("wkT", [HID, HL * D], F32R, kind="ExternalInput")
    tn["wvT"] = nc.dram_tensor("wvT", [HID, HL * D], F32R, kind="ExternalInput")
    tn["woT"] = nc.dram_tensor("woT", [HID, HID], F32R, kind="ExternalInput")
    tn["biasT"] = nc.dram_tensor("biasT", [128, HL * TT], F32, kind="ExternalInput")
    tn["y"] = nc.dram_tensor("y", [ROWS, HID], F32, kind="ExternalOutput")
    tn["sums"] = nc.dram_tensor("sums_scratch", [HL, 5, S], F32R)
    tn["qg1"] = nc.dram_tensor("qg1_spill", [D, 3 * S], F32R)
    tn["kg1"] = nc.dram_tensor("kg1_spill", [D, 3 * S], F32R)
    taps = {}
    if debug_taps:
        taps["qt"] = nc.dram_tensor("dbg_qt", [D, HL * S], F32R, kind="ExternalOutput")
        taps["kt"] = nc.dram_tensor("dbg_kt", [D, HL * S], F32R, kind="ExternalOutput")
        taps["vp"] = nc.dram_tensor("dbg_vp", [128, TT * VW], F32R, kind="ExternalOutput")
        taps["otn"] = nc.dram_tensor("dbg_otn", [D, HL * S], F32R, kind="ExternalOutput")

    with tile.TileContext(nc) as tc:
        for _ in range(n_iters):
            _emit_iter(nc, tc, tn, scale, taps)
    nc.compile()
    return nc


def _emit_iter(nc, tc, tn, scale, taps):
    xT, wqT, wkT, wvT, woT = tn["xT"], tn["wqT"], tn["wkT"], tn["wvT"], tn["woT"]
    biasT, y, sums_dram = tn["biasT"], tn["y"], tn["sums"]
    qg1, kg1 = tn["qg1"], tn["kg1"]
    debug_taps = bool(taps)

    with ExitStack() as ctx:
        pa = ctx.enter_context(tc.tile_pool(name="pa", bufs=1))
        vp = pa.tile([128, TT * VW], F32R, name="vp")  # V'; col = tt*VW + h*101 + d
        bias_sb = pa.tile([128, HL * TT], F32, name="bias_sb")
        ones1 = pa.tile([1, D], F32R, name="ones1")
        nc.sync.dma_start(out=bias_sb, in_=biasT.ap())
        nc.vector.memset(ones1.bitcast(F32), 1.0)
        nc.vector.tensor_copy(out=ones1, in_=ones1.bitcast(F32))

        with tc.tile_pool(name="pqk3", bufs=1) as pqk3:
            qt3 = [None, None]
            kt3 = [None, None]

            # ============ P1: x^T resident; V' + both Q/K projections ======
            with tc.tile_pool(name="pxt", bufs=1) as pxt:
                xt = pxt.tile([CK, CCH * S], F32R, name="xt")

                # ones cols pre-set; V cols overwritten. DVE Memset can't
                # write f32r directly: memset an f32 view, then retag the
                # ones columns (stride-101 view) with a rounding self-copy.
                nc.vector.memset(vp.bitcast(F32), 1.0)
                ones_v = vp.rearrange("p (n k) -> p n k", k=101)[:, :, 100]
                nc.vector.tensor_copy(out=ones_v, in_=ones_v.bitcast(F32))

                # ---- V' projection; x^T chunks stream in on first pass ----
                with tc.tile_pool(name="psv", bufs=8, space="PSUM") as psv, \
                     tc.tile_pool(name="pwv", bufs=3) as pwv:
                    first = True
                    for jh in range(2):            # j half: heads 3jh..3jh+2
                        for tg in range(2):        # t-tile groups of 8
                            accs = [psv.tile([128, 300], F32, tag="vacc",
                                             name="vacc") for _ in range(8)]
                            for c in range(CCH):
                                if first:
                                    nc.sync.dma_start(
                                        out=xt[:, c * S : (c + 1) * S],
                                        in_=xT.ap()[c * CK : (c + 1) * CK, :])
                                    _absorb(nc, xt[:, c * S : (c + 1) * S])
                                wv_c = pwv.tile([CK, 300], F32R, tag="wv",
                                                name="wv_c")
                                nc.sync.dma_start(
                                    out=wv_c,
                                    in_=wvT.ap()[c * CK : (c + 1) * CK,
                                                 jh * 300 : (jh + 1) * 300])
                                _absorb(nc, wv_c)
                                for tl in range(8):
                                    t0 = c * S + (tg * 8 + tl) * 128
                                    _mm(nc, accs[tl][:, :],
                                        xt[:, t0 : t0 + 128], wv_c[:, :],
                                        start=(c == 0), stop=(c == CCH - 1))
                            first = False
                            for tl in range(8):
                                tt = tg * 8 + tl
                                for hh in range(3):
                                    h = jh * 3 + hh
                                    c0 = tt * VW + h * 101
                                    nc.vector.tensor_copy(
                                        out=vp[:, c0 : c0 + D],
                                        in_=accs[tl][:, hh * D : (hh + 1) * D])

                # ---- Q/K projections for both groups ----------------------
                with tc.tile_pool(name="pwqk", bufs=3) as pwqk, \
                     tc.tile_pool(name="pstg", bufs=3) as pstg:
                    qt3[0], kt3[0] = _proj_qk(nc, tc, pqk3, pwqk, wqT, wkT, 0,
                                              xt, None, None, None)
                    if debug_taps:
                        nc.sync.dma_start(out=taps["qt"].ap()[:, 0 : 3 * S], in_=qt3[0])
                        nc.sync.dma_start(out=taps["kt"].ap()[:, 0 : 3 * S], in_=kt3[0])
                    _proj_qk(nc, tc, pqk3, pwqk, wqT, wkT, 1,
                             xt, pstg, qg1, kg1)

            # ============ attention (+inline norm), then WO ================
            with tc.tile_pool(name="pot", bufs=1, side="right") as pot:
                ot = pot.tile([D + 1, HL * S], F32R, name="ot")

                with tc.tile_pool(name="ppt", bufs=3) as ppt, \
                     tc.tile_pool(name="pnr", bufs=2) as pnr, \
                     tc.tile_pool(name="psn", bufs=2, space="PSUM") as psn:
                    _attend(nc, tc, 0, qt3[0], kt3[0], vp, bias_sb, ot,
                            sums_dram, ppt, psn, ones1, pnr, scale)

                    # read back group 1's Q/K (spilled to DRAM in P1)
                    qt3[1] = pqk3.tile([D, 3 * S], F32R, tag="qt3", name="qt")
                    kt3[1] = pqk3.tile([D, 3 * S], F32R, tag="kt3", name="kt")
                    for j in range(3):
                        nc.sync.dma_start(
                            out=qt3[1][:, j * S : (j + 1) * S],
                            in_=qg1.ap()[:, j * S : (j + 1) * S])
                        nc.sync.dma_start(
                            out=kt3[1][:, j * S : (j + 1) * S],
                            in_=kg1.ap()[:, j * S : (j + 1) * S])
                    if debug_taps:
                        nc.sync.dma_start(out=taps["qt"].ap()[:, 3 * S : 6 * S], in_=qt3[1])
                        nc.sync.dma_start(out=taps["kt"].ap()[:, 3 * S : 6 * S], in_=kt3[1])

                    _attend(nc, tc, 1, qt3[1], kt3[1], vp, bias_sb, ot,
                            sums_dram, ppt, psn, ones1, pnr, scale)

                if debug_taps:
                    nc.sync.dma_start(out=taps["vp"].ap(), in_=vp)
                    nc.sync.dma_start(out=taps["otn"].ap(), in_=ot[0:D, :])

                # ============ output projection ============================
                # comb^T chunk m at free pos r equals ot[:, r*12 + m]
                ot_r = ot[0:D, :].rearrange("p (r m) -> p r m", m=HID // D)
                with tc.tile_pool(name="pwo", bufs=3) as pwo, \
                     tc.tile_pool(name="pyb", bufs=4) as pyb, \
                     tc.tile_pool(name="psy", bufs=8, space="PSUM") as psy:
                    for jb in range(3):
                        pys = [psy.tile([128, 400], F32, tag="py", name="py")
                               for _ in range(8)]
                        for m in range(HID // D):
                            wo_s = pwo.tile([D, 400], F32R, tag="wo", name="wo_s")
                            nc.sync.dma_start(
                                out=wo_s,
                                in_=woT.ap()[m * D : (m + 1) * D,
                                             jb * 400 : (jb + 1) * 400])
                            _absorb(nc, wo_s)
                            for rt in range(8):
                                _mm(nc, pys[rt][:, :],
                                    ot_r[:, rt * 128 : (rt + 1) * 128, m],
                                    wo_s[:, :],
                                    start=(m == 0), stop=(m == HID // D - 1))
                        for rt in range(8):
                            ysb = pyb.tile([128, 400], F32, tag="ysb", name="ysb")
                            nc.vector.tensor_copy(out=ysb, in_=pys[rt][:, :])
                            nc.sync.dma_start(
                                out=y.ap()[rt * 128 : (rt + 1) * 128,
                                           jb * 400 : (jb + 1) * 400],
                                in_=ysb)


def _proj_qk(nc, tc, pqk3, pwqk, wqT, wkT, g2, xt, pstg, qg1, kg1):
    """Project Q^T/K^T for 3 heads of group g2 from resident x^T.

    g2=0: into [D, 3*S] SBUF tiles (returned). g2=1: spilled to DRAM via
    staging tiles (qg1/kg1), read back during attention of group 0.
    """
    if g2 == 0:
        qt = pqk3.tile([D, 3 * S], F32R, tag="qt3", name="qt")
        kt = pqk3.tile([D, 3 * S], F32R, tag="kt3", name="kt")
    with tc.tile_pool(name=f"psp{g2}", bufs=4, space="PSUM") as psp:
        for j in range(3):
            h = g2 * 3 + j
            qacc = [psp.tile([D, 512], F32, tag="qacc", name="qacc")
                    for _ in range(4)]
            kacc = [psp.tile([D, 512], F32, tag="kacc", name="kacc")
                    for _ in range(4)]
            for c in range(CCH):
                wq_s = pwqk.tile([CK, D], F32R, tag="wq", name="wq_s")
                wk_s = pwqk.tile([CK, D], F32R, tag="wk", name="wk_s")
                nc.sync.dma_start(
                    out=wq_s, in_=wqT.ap()[c * CK : (c + 1) * CK,
                                           h * D : (h + 1) * D])
                nc.sync.dma_start(
                    out=wk_s, in_=wkT.ap()[c * CK : (c + 1) * CK,
                                           h * D : (h + 1) * D])
                _absorb(nc, wq_s)
                _absorb(nc, wk_s)
                for sb in range(4):
                    _mm(nc, qacc[sb][:, :], wq_s[:, :],
                        xt[:, c * S + sb * 512 : c * S + (sb + 1) * 512],
                        start=(c == 0), stop=(c == CCH - 1))
                    _mm(nc, kacc[sb][:, :], wk_s[:, :],
                        xt[:, c * S + sb * 512 : c * S + (sb + 1) * 512],
                        start=(c == 0), stop=(c == CCH - 1))
            for sb in range(4):
                if g2 == 0:
                    nc.vector.tensor_copy(
                        out=qt[:, j * S + sb * 512 : j * S + (sb + 1) * 512],
                        in_=qacc[sb][:, :])
                    nc.vector.tensor_copy(
                        out=kt[:, j * S + sb * 512 : j * S + (sb + 1) * 512],
                        in_=kacc[sb][:, :])
                else:
                    for acc, dest in ((qacc[sb], qg1), (kacc[sb], kg1)):
                        st = pstg.tile([D, 512], F32R, tag="stg", name="st")
                        nc.vector.tensor_copy(out=st, in_=acc[:, :])
                        nc.sync.dma_start(
                            out=dest.ap()[:, j * S + sb * 512 : j * S + (sb + 1) * 512],
                            in_=st)
    if g2 == 0:
        return qt, kt
    return None, None


def _attend(nc, tc, g2, qt, kt, vp, bias_sb, ot, sums_dram, ppt, psn,
            ones1, pnr, scale):
    """scores^T -> exp -> PV for 3 heads; per-head normalization inline."""
    with tc.tile_pool(name=f"psa{g2}", bufs=1, space="PSUM") as psa:
        for j in range(3):
            h = g2 * 3 + j
            for sh in range(2):
                s0 = sh * 1024
                po = psa.tile([D + 1, 1024], F32, tag="ps_o", name="po")
                for tt in range(TT):
                    ss = psa.tile([128, 1024], F32, tag="ps_s", name="ss",
                                  bufs=2)
                    for sbb in range(2):
                        _mm(nc, ss[:, sbb * 512 : (sbb + 1) * 512],
                            kt[:, j * S + tt * 128 : j * S + (tt + 1) * 128],
                            qt[:, j * S + s0 + sbb * 512 : j * S + s0 + (sbb + 1) * 512],
                            start=True, stop=True)
                    pt = ppt.tile([128, 1024], F32R, tag="pt", name="pt")
                    nc.scalar.activation(
                        out=pt, in_=ss[:, :], func=EXP,
                        bias=bias_sb[:, h * TT + tt : h * TT + tt + 1],
                        scale=scale)
                    for sbb in range(2):
                        _mm(nc, po[0 : D + 1, sbb * 512 : (sbb + 1) * 512],
                            vp[:, tt * VW + h * 101 : tt * VW + h * 101 + 101],
                            pt[:, sbb * 512 : (sbb + 1) * 512],
                            start=(tt == 0), stop=(tt == TT - 1))
                nc.vector.tensor_copy(
                    out=ot[0 : D + 1, h * S + s0 : h * S + s0 + 1024],
                    in_=po[0 : D + 1, :])
                nc.sync.dma_start(
                    out=sums_dram.ap()[h, :, s0 : s0 + 1024],
                    in_=ot[96 : 101, h * S + s0 : h * S + s0 + 1024])

            # ---- inline per-head softmax normalization (fills bubbles) ----
            srow = pnr.tile([1, S], F32R, tag="srow", name="srow")
            rrow = pnr.tile([1, S], F32R, tag="rrow", name="rrow")
            nc.sync.dma_start(out=srow, in_=sums_dram.ap()[h, 4, :])
            with nc.allow_low_precision(reason="f32r recip of softmax sums"):
                nc.vector.reciprocal(out=rrow, in_=srow)
            for blk in range(4):
                pb = psn.tile([D, 512], F32, tag="pb", name="pb")
                _mm(nc, pb[:, :], ones1[0:1, :],
                    rrow[0:1, blk * 512 : (blk + 1) * 512],
                    start=True, stop=True)
                sl = ot[0:D, h * S + blk * 512 : h * S + (blk + 1) * 512]
                nc.vector.tensor_mul(sl, sl, pb[:, :])


def make_core_inputs(x, alibi, attention_mask, wq, wk, wv, wo, layer_index):
    li = int(np.asarray(layer_index))
    inv = np.float32(1.0 / (li + 1))
    woT = np.ascontiguousarray(np.asarray(wo, dtype=np.float32).T)
    xTs = [np.ascontiguousarray(np.asarray(x[b], dtype=np.float32).T)
           for b in range(B)]
    wts = []
    for g in range(HG):
        sl = slice(g * HL * D, (g + 1) * HL * D)
        wts.append(tuple(
            np.ascontiguousarray(np.asarray(w, dtype=np.float32)[sl, :].T)
            for w in (wq, wk, wv)))
    in_maps = []
    for b in range(B):
        for g in range(HG):
            a = np.asarray(alibi, dtype=np.float32)[
                b * H + g * HL : b * H + (g + 1) * HL, 0, :]      # (6, S)
            msk = np.asarray(attention_mask, dtype=np.float32)[b, 0, 0, :S]
            bias = a * inv + msk[None, :]                          # (6, S)
            biasT = np.ascontiguousarray(
                bias.reshape(HL, TT, 128).transpose(2, 0, 1).reshape(128, HL * TT))
            wqT, wkT, wvT = wts[g]
            in_maps.append({
                "xT": xTs[b], "wqT": wqT, "wkT": wkT, "wvT": wvT,
                "woT": woT, "biasT": biasT,
            })
    scale = float(np.float32(np.sqrt(np.float32(D))) * inv)
    return in_maps, scale


def run(trace=False, **inputs):
    in_maps, scale = make_core_inputs(**inputs)
    nc = build_program(scale)
    res = run_bass_kernel_spmd(nc, in_maps, core_ids=list(range(B * HG)),
                               trace=trace)
    out = np.empty((B, S, HID), dtype=np.float32)
    for b in range(B):
        for g in range(HG):
            out[b, g * ROWS : (g + 1) * ROWS, :] = res.results[b * HG + g]["y"]
    return out, res


def kernel(**inputs) -> np.ndarray:
    out, _ = run(trace=False, **inputs)
    return out



# revision 23
# speedup vs baseline: 3.0186x; 3.0186x over previous
"""Trainium2 Bass kernel for nn_Attention_71966472012100.

Multi-head attention layer (dense_transformer), B=4, S=2048, H=12, D=100,
HID=1200, with the reference's bug-faithful head-mixing reshape before the
output projection.

Sharding: 8 cores = data-parallel over batch (4) x tensor-parallel over head
groups (2 groups of 6 heads). Because the reference reshapes (b, h, s, d) ->
(b, s, h*d) WITHOUT permuting heads back, head-group g's attention output
occupies exactly rows [g*1024, (g+1)*1024) of the reshaped activation.
Each core produces 1024 complete rows of the final output; no cross-core
communication.

Layouts (all transposed so no transposes are ever needed):
  QT/KT: [d, s] per head (f32r - the exp amplifies q/k quantization error,
         so this path stays at FP22 precision; rest of kernel is bf16)
  scores^T: [t, s] in PSUM (softmax axis t on partitions; alibi+mask are a
         per-partition bias fused into the ACT exp)
  V': [t, d|ones] bf16 (ones column makes PV also produce softmax sums)
  O^T: [d, h*S+s] bf16 (strided column views are the WO lhsT tiles)

Schedule: the Tile scheduler is dependency-driven, so the emission order
below just sets priorities. Structure:
  - x^T half-chunks stream in; V' (first tile-groups) and head-0/1 Q/K
    projections consume chunks as they arrive (PE busy during the DMA).
  - remaining V' groups, then Q/K for heads 2..5 into ROTATING 2-deep
    per-head buffers. Projection of head h+1 runs during attention of head
    h, filling the PE slack under the ACT(exp)-bound attention phase.
  - attention per head: QK -> exp(ACT) -> PV into per-512-block PSUM tiles;
    softmax normalization evacuates each block promptly (sums row + raw O^T
    copy free the bank), recip + rank-1 broadcast + in-place bf16 multiply.
  - WO from strided O^T views with the whole wo resident in SBUF (bf16).
PSUM budget (8 banks): ss 2x2 + po 2x1 + shared transient 2 (proj accs and
norm pb alternate in the same slots).
"""

import numpy as np
import ml_dtypes
from contextlib import ExitStack

import concourse.bass as bass
import concourse.tile as tile
from concourse import bacc
from concourse import mybir
from concourse.bass_utils import run_bass_kernel_spmd

F32 = mybir.dt.float32
F32R = mybir.dt.float32r
BF16 = mybir.dt.bfloat16
EXP = mybir.ActivationFunctionType.Exp

B, S, H, D, HID = 4, 2048, 12, 100, 1200
HG = 2                # head groups (tensor parallel)
HL = H // HG          # 6 heads per core
ROWS = S * HL * D // HID   # 1024 output rows per core
CK, CCH = 120, 10     # contraction chunking of HID
TT = S // 128         # 16 key tiles
VW = HL * (D + 1)     # 606: V' row width per t-tile (d cols + ones col per head)
NM = HID // D         # 12 output-projection contraction chunks


def _mm(nc, out, lhsT, rhs, **kw):
    nc.tensor.matmul(out, lhsT, rhs, **kw)


def _absorb(nc, ap):
    """PE-side observation of a freshly DMA'd tile (absorbs a DMA wait in a
    throwaway 1-column LDWEIGHTS ahead of the real matmuls)."""
    bb = ap.bitcast(BF16)
    nc.tensor.ldweights(bb[:, 0:1])


def build_program(scale: float, debug_taps: bool = False, n_iters: int = 1):
    nc = bacc.Bacc("TRN2", target_bir_lowering=False, debug=False)

    tn = {}
    tn["xT"] = nc.dram_tensor("xT", [HID, S], F32R, kind="ExternalInput")
    # per-head packed q||k weights: [120, c(10) * 200] f32r
    tn["wqk"] = nc.dram_tensor("wqk", [HL, CK, CCH * 2 * D], F32R,
                               kind="ExternalInput")
    # packed v weights: [120, c(10) * 600] bf16
    tn["wv"] = nc.dram_tensor("wv", [CK, CCH * HL * D], BF16,
                              kind="ExternalInput")
    # packed wo: [100, m(12) * 1200] bf16
    tn["wo"] = nc.dram_tensor("wo", [D, NM * HID], BF16, kind="ExternalInput")
    tn["biasT"] = nc.dram_tensor("biasT", [128, HL * TT], F32,
                                 kind="ExternalInput")
    tn["y"] = nc.dram_tensor("y", [ROWS, HID], F32, kind="ExternalOutput")

    with tile.TileContext(nc) as tc:
        for _ in range(n_iters):
            _emit_iter(nc, tc, tn, scale)
    nc.compile()
    return nc


def _emit_iter(nc, tc, tn, scale):
    xT, wqk, wv, wo, biasT, y = (tn["xT"], tn["wqk"], tn["wv"], tn["wo"],
                                 tn["biasT"], tn["y"])

    with ExitStack() as ctx:
        pa = ctx.enter_context(tc.tile_pool(name="pa", bufs=1))
        vp = pa.tile([128, TT * VW], BF16, name="vp")
        ot = pa.tile([D, HL * S], BF16, name="ot")
        bias_sb = pa.tile([128, HL * TT], F32, name="bias_sb")
        ones1 = pa.tile([1, D], F32R, name="ones1")
        wv_sb = pa.tile([CK, CCH * HL * D], BF16, name="wv_sb")

        pqk = ctx.enter_context(tc.tile_pool(name="pqk", bufs=2))
        pwqk = ctx.enter_context(tc.tile_pool(name="pwqk", bufs=2))
        ppt = ctx.enter_context(tc.tile_pool(name="ppt", bufs=3))
        pnr = ctx.enter_context(tc.tile_pool(name="pnr", bufs=2))

        qk_tiles = {}
        w_tiles = {}

        def emit_wqk_dma(h):
            w_sb = pwqk.tile([CK, CCH * 2 * D], F32R, tag="wqk", name="w_sb")
            nc.sync.dma_start(out=w_sb, in_=wqk.ap()[h])
            _absorb(nc, w_sb)
            w_tiles[h] = w_sb

        def emit_proj(h):
            """Q/K projection of head h from resident x^T into rotating
            [D, S] f32r tiles. Uses the 2-slot transient PSUM pool."""
            w_sb = w_tiles.pop(h)
            qt = pqk.tile([D, S], F32R, tag="qt", name="qt")
            kt = pqk.tile([D, S], F32R, tag="kt", name="kt")
            for sb in range(4):
                for qk, dest in ((0, qt), (1, kt)):
                    acc = ptr.tile([D, 512], F32, tag="tr", name="acc")
                    for c in range(CCH):
                        _mm(nc, acc[:, :],
                            w_sb[:, c * 2 * D + qk * D: c * 2 * D + (qk + 1) * D],
                            xt[:, c * S + sb * 512: c * S + (sb + 1) * 512],
                            start=(c == 0), stop=(c == CCH - 1))
                    nc.vector.tensor_copy(
                        out=dest[:, sb * 512:(sb + 1) * 512], in_=acc[:, :])
            qk_tiles[h] = (qt, kt)

        def emit_vprime(tts):
            """V' for t-tiles `tts` (<=3 at a time), both jh head-halves.
            The PE can't mix f32r x with bf16 wv, so x tiles are converted
            to bf16 via small rolling DVE copies (the V path tolerates bf16
            x). accs: 2*len(tts) banks + 2 transient <= 8."""
            w = len(tts) * 128
            with tc.tile_pool(name=f"psv{tts[0]}", bufs=2 * len(tts),
                              space="PSUM") as psv, \
                 tc.tile_pool(name=f"pxb{tts[0]}", bufs=2) as pxb:
                accs = {(jh, t): psv.tile([128, 3 * D], F32, tag="vacc",
                                          name="vacc")
                        for jh in range(2) for t in tts}
                for c in range(CCH):
                    xb = pxb.tile([CK, w], BF16, tag="xb", name="xb")
                    nc.vector.tensor_copy(
                        out=xb, in_=xt[:, c * S + tts[0] * 128:
                                       c * S + tts[0] * 128 + w])
                    for jh in range(2):
                        for i, t in enumerate(tts):
                            _mm(nc, accs[(jh, t)][:, :],
                                xb[:, i * 128:(i + 1) * 128],
                                wv_sb[:, c * HL * D + jh * 3 * D:
                                      c * HL * D + (jh + 1) * 3 * D],
                                start=(c == 0), stop=(c == CCH - 1))
                for (jh, t), acc in accs.items():
                    dst = vp[:, t * VW + jh * 3 * (D + 1):
                             t * VW + (jh + 1) * 3 * (D + 1)]
                    dst3 = dst.rearrange("p (h e) -> p h e", e=D + 1)
                    nc.any.tensor_copy(out=dst3[:, :, 0:D],
                                       in_=acc.rearrange(
                                           "p (h d) -> p h d", d=D))

        pending_norm = []

        def emit_norm_finish():
            """recip + rank-1 broadcast + in-place multiply for a previously
            evacuated (h, sh) block pair. Deferred one s-half window so the
            pb matmuls sit behind the next tt-loop in the static PE order
            (they depend on the slow DVE chain)."""
            if not pending_norm:
                return
            for col, srow in pending_norm.pop(0):
                srow1 = pnr.tile([1, 512], F32R, tag="srow1", name="srow1")
                nc.sync.dma_start(out=srow1, in_=srow[4:5, :])
                rrow = pnr.tile([1, 512], F32R, tag="rrow", name="rrow")
                with nc.allow_low_precision(reason="f32r recip of sums"):
                    nc.vector.reciprocal(out=rrow, in_=srow1)
                pb = ptr.tile([D, 512], F32, tag="tr", name="pb")
                _mm(nc, pb[:, :], ones1[0:1, :], rrow[0:1, :],
                    start=True, stop=True)
                nc.vector.tensor_mul(ot[:, col:col + 512],
                                     ot[:, col:col + 512], pb[:, :])

        def emit_attention_sh(h, sh, qt, kt):
            s0 = sh * 1024
            pos = [ppo.tile([D + 1, 512], F32, tag="po", name="po")
                   for _ in range(2)]
            for tt in range(TT):
                ss = pss.tile([128, 1024], F32, tag="ss", name="ss")
                for sbb in range(2):
                    _mm(nc, ss[:, sbb * 512:(sbb + 1) * 512],
                        kt[:, tt * 128:(tt + 1) * 128],
                        qt[:, s0 + sbb * 512: s0 + (sbb + 1) * 512],
                        start=True, stop=True)
                pt = ppt.tile([128, 1024], BF16, tag="pt", name="pt")
                nc.scalar.activation(
                    out=pt, in_=ss[:, :], func=EXP,
                    bias=bias_sb[:, h * TT + tt: h * TT + tt + 1],
                    scale=scale)
                for sbb in range(2):
                    _mm(nc, pos[sbb][:, :],
                        vp[:, tt * VW + h * (D + 1):
                           tt * VW + (h + 1) * (D + 1)],
                        pt[:, sbb * 512:(sbb + 1) * 512],
                        start=(tt == 0), stop=(tt == TT - 1))
            # evacuation: sums row + raw O^T copy free each po bank promptly
            grp = []
            for sbb in range(2):
                po = pos[sbb]
                col = h * S + s0 + sbb * 512
                srow = pnr.tile([5, 512], F32R, tag="srow", name="srow",
                                bufs=4)
                nc.vector.tensor_copy(out=srow, in_=po[96:D + 1, :])
                nc.vector.tensor_copy(out=ot[:, col:col + 512],
                                      in_=po[0:D, :])
                grp.append((col, srow))
            pending_norm.append(grp)

        # ================= emission (priority) order =================
        nc.sync.dma_start(out=bias_sb, in_=biasT.ap())
        emit_wqk_dma(0)
        wv_sb3 = wv_sb.rearrange("p (c w) -> p c w", w=HL * D)
        wv_dr3 = wv.ap().rearrange("p (c w) -> p c w", w=HL * D)
        nc.sync.dma_start(out=wv_sb3[:, :, 0:3 * D], in_=wv_dr3[:, :, 0:3 * D])
        emit_wqk_dma(1)
        nc.sync.dma_start(out=wv_sb3[:, :, 3 * D:HL * D],
                          in_=wv_dr3[:, :, 3 * D:HL * D])
        nc.vector.memset(ones1.bitcast(F32), 1.0)
        nc.vector.tensor_copy(out=ones1, in_=ones1.bitcast(F32))
        # ones cols pre-set; V cols overwritten by the V' copies below.
        nc.vector.memset(vp, 1.0)

        with tc.tile_pool(name="pxt", bufs=1) as pxt:
            xt = pxt.tile([CK, CCH * S], F32R, name="xt")
            for c2 in range(2 * CCH):
                nc.sync.dma_start(
                    out=xt[:, c2 * 1024:(c2 + 1) * 1024],
                    in_=xT.ap()[(c2 // 2) * CK:(c2 // 2 + 1) * CK,
                                (c2 % 2) * 1024:((c2 % 2) + 1) * 1024])
                _absorb(nc, xt[:, c2 * 1024:(c2 + 1) * 1024])

            # shared 2-bank transient PSUM: proj accs and norm pb rotate
            with tc.tile_pool(name="ptr", bufs=2, space="PSUM") as ptr:
                # consume x chunks as they arrive: V' + head-0/1 proj
                emit_vprime(range(0, 3))
                emit_proj(0)
                emit_vprime(range(3, 6))
                emit_proj(1)
                emit_vprime(range(6, 9))
                emit_vprime(range(9, 12))
                emit_vprime(range(12, 15))
                emit_vprime(range(15, 16))

                with tc.tile_pool(name="pss", bufs=2, space="PSUM") as pss, \
                     tc.tile_pool(name="ppo", bufs=2, space="PSUM") as ppo:
                    for h in range(6):
                        if h >= 2:
                            emit_wqk_dma(h)
                            emit_proj(h)
                        qt, kt = qk_tiles.pop(h)
                        for sh in range(2):
                            emit_attention_sh(h, sh, qt, kt)
                            if len(pending_norm) > 1:
                                emit_norm_finish()
                    emit_norm_finish()
                    emit_norm_finish()

        # ============ output projection (wo resident in SBUF) ============
        pwo = ctx.enter_context(tc.tile_pool(name="pwo", bufs=1))
        pyb = ctx.enter_context(tc.tile_pool(name="pyb", bufs=4))
        wo_sb = pwo.tile([D, NM * HID], BF16, name="wo_sb")
        nc.sync.dma_start(out=wo_sb, in_=wo.ap())
        _absorb(nc, wo_sb)
        ot_r = ot.rearrange("p (r m) -> p r m", m=NM)
        with tc.tile_pool(name="psy", bufs=8, space="PSUM") as psy:
            for jb in range(3):
                pys = [psy.tile([128, 400], F32, tag="py", name="py")
                       for _ in range(8)]
                for m in range(NM):
                    for rt in range(8):
                        _mm(nc, pys[rt][:, :],
                            ot_r[:, rt * 128:(rt + 1) * 128, m],
                            wo_sb[:, m * HID + jb * 400:
                                  m * HID + (jb + 1) * 400],
                            start=(m == 0), stop=(m == NM - 1))
                for rt in range(8):
                    ysb = pyb.tile([128, 400], F32, tag="ysb", name="ysb")
                    nc.any.tensor_copy(out=ysb, in_=pys[rt][:, :])
                    nc.sync.dma_start(
                        out=y.ap()[rt * 128:(rt + 1) * 128,
                                   jb * 400:(jb + 1) * 400],
                        in_=ysb)


def make_core_inputs(x, alibi, attention_mask, wq, wk, wv, wo, layer_index):
    li = int(np.asarray(layer_index))
    inv = np.float32(1.0 / (li + 1))
    f32 = np.float32
    bf16 = ml_dtypes.bfloat16

    xTs = [np.ascontiguousarray(np.asarray(x[b], dtype=f32).T)
           for b in range(B)]

    # packed wo: wo_pk[d, m*1200 + n] = wo.T[m*100+d, n]
    woT = np.asarray(wo, dtype=f32).T                       # [1200, 1200]
    wo_pk = np.ascontiguousarray(
        woT.reshape(NM, D, HID).transpose(1, 0, 2).reshape(D, NM * HID)
    ).astype(bf16)

    per_group = []
    for g in range(HG):
        sl = slice(g * HL * D, (g + 1) * HL * D)
        # wqk[h, p, c*200 + qk*100 + d] = w{q,k}[g*600 + h*100 + d, c*120 + p]
        wq_g = np.asarray(wq, dtype=f32)[sl, :]             # [600, 1200]
        wk_g = np.asarray(wk, dtype=f32)[sl, :]
        wqk_pk = np.empty((HL, CK, CCH * 2 * D), dtype=f32)
        for h in range(HL):
            qh = wq_g[h * D:(h + 1) * D, :].T               # [1200, 100]
            kh = wk_g[h * D:(h + 1) * D, :].T
            both = np.concatenate(
                [qh.reshape(CCH, CK, D), kh.reshape(CCH, CK, D)],
                axis=2)                                     # [10, 120, 200]
            wqk_pk[h] = both.transpose(1, 0, 2).reshape(CK, CCH * 2 * D)
        # wv_pk[p, c*600 + col] = wv[g*600 + col, c*120 + p]
        wv_g = np.asarray(wv, dtype=f32)[sl, :].T           # [1200, 600]
        wv_pk = np.ascontiguousarray(
            wv_g.reshape(CCH, CK, HL * D).transpose(1, 0, 2)
            .reshape(CK, CCH * HL * D)).astype(bf16)
        per_group.append((np.ascontiguousarray(wqk_pk), wv_pk))

    in_maps = []
    for b in range(B):
        for g in range(HG):
            a = np.asarray(alibi, dtype=f32)[
                b * H + g * HL: b * H + (g + 1) * HL, 0, :]      # (6, S)
            msk = np.asarray(attention_mask, dtype=f32)[b, 0, 0, :S]
            bias = a * inv + msk[None, :]                        # (6, S)
            biasT = np.ascontiguousarray(
                bias.reshape(HL, TT, 128).transpose(2, 0, 1)
                .reshape(128, HL * TT))
            wqk_pk, wv_pk = per_group[g]
            in_maps.append({
                "xT": xTs[b], "wqk": wqk_pk, "wv": wv_pk,
                "wo": wo_pk, "biasT": biasT,
            })
    scale = float(np.float32(np.sqrt(np.float32(D))) * inv)
    return in_maps, scale


def run(trace=False, **inputs):
    in_maps, scale = make_core_inputs(**inputs)
    nc = build_program(scale)
    res = run_bass_kernel_spmd(nc, in_maps, core_ids=list(range(B * HG)),
                               trace=trace)
    out = np.empty((B, S, HID), dtype=np.float32)
    for b in range(B):
        for g in range(HG):
            out[b, g * ROWS:(g + 1) * ROWS, :] = res.results[b * HG + g]["y"]
    return out, res


def kernel(**inputs) -> np.ndarray:
    out, _ = run(trace=False, **inputs)
    return out


# revision 29
# speedup vs baseline: 3.9147x; 1.2968x over previous
"""Trainium2 Bass kernel for nn_Attention_71966472012100.

Multi-head attention layer (dense_transformer), B=4, S=2048, H=12, D=100,
HID=1200, with the reference's bug-faithful head-mixing reshape before the
output projection.

Sharding: 8 cores = data-parallel over batch (4) x tensor-parallel over head
groups (2 groups of 6 heads). Because the reference reshapes (b, h, s, d) ->
(b, s, h*d) WITHOUT permuting heads back, head-group g's attention output
occupies exactly rows [g*1024, (g+1)*1024) of the reshaped activation.
Each core produces 1024 complete rows of the final output; no cross-core
communication.

Layouts (all transposed so no transposes are ever needed):
  QT/KT: [d, s] per head (f32r - the exp amplifies q/k quantization error,
         so this path stays at FP22 precision; rest of kernel is bf16)
  scores^T: [t, s] in PSUM (softmax axis t on partitions; alibi+mask are a
         per-partition bias fused into the ACT exp)
  V': [t, d|ones] bf16 (ones column makes PV also produce softmax sums)
  O^T: [d, h*S+s] bf16 (strided column views are the WO lhsT tiles)

Schedule: the Tile scheduler is dependency-driven, so the emission order
below just sets priorities. Structure:
  - x^T half-chunks stream in; V' (first tile-groups) and head-0/1 Q/K
    projections consume chunks as they arrive (PE busy during the DMA).
  - remaining V' groups, then Q/K for heads 2..5 into ROTATING 2-deep
    per-head buffers. Projection of head h+1 runs during attention of head
    h, filling the PE slack under the ACT(exp)-bound attention phase.
  - attention per head: QK -> exp(ACT) -> PV into per-512-block PSUM tiles;
    softmax normalization evacuates each block promptly (sums row + raw O^T
    copy free the bank), recip + rank-1 broadcast + in-place bf16 multiply.
  - WO from strided O^T views with the whole wo resident in SBUF (bf16).
PSUM budget (8 banks): ss 2x2 + po 2x1 + shared transient 2 (proj accs and
norm pb alternate in the same slots).
"""

import numpy as np
import ml_dtypes
from contextlib import ExitStack

import concourse.bass as bass
import concourse.tile as tile
from concourse import bacc
from concourse import mybir
from concourse.bass_utils import run_bass_kernel_spmd

F32 = mybir.dt.float32
F32R = mybir.dt.float32r
BF16 = mybir.dt.bfloat16
EXP = mybir.ActivationFunctionType.Exp

B, S, H, D, HID = 4, 2048, 12, 100, 1200
HG = 2                # head groups (tensor parallel)
HL = H // HG          # 6 heads per core
ROWS = S * HL * D // HID   # 1024 output rows per core
CK, CCH = 120, 10     # contraction chunking of HID
TT = S // 128         # 16 key tiles
VW = HL * (D + 1)     # 606: V' row width per t-tile (d cols + ones col per head)
NM = HID // D         # 12 output-projection contraction chunks


def _mm(nc, out, lhsT, rhs, **kw):
    nc.tensor.matmul(out, lhsT, rhs, **kw)


def _absorb(nc, ap):
    """PE-side observation of a freshly DMA'd tile (absorbs a DMA wait in a
    throwaway 1-column LDWEIGHTS ahead of the real matmuls)."""
    bb = ap.bitcast(BF16)
    nc.tensor.ldweights(bb[:, 0:1])


def build_program(scale: float, debug_taps: bool = False, n_iters: int = 1):
    nc = bacc.Bacc("TRN2", target_bir_lowering=False, debug=False)

    tn = {}
    tn["xT"] = nc.dram_tensor("xT", [HID, S], F32R, kind="ExternalInput")
    # per-head packed q||k weights: [120, c(10) * 200] f32r
    tn["wqk"] = nc.dram_tensor("wqk", [HL, CK, CCH * 2 * D], F32R,
                               kind="ExternalInput")
    # packed v weights: [120, c(10) * 600] f32r
    tn["wv"] = nc.dram_tensor("wv", [CK, CCH * HL * D], F32R,
                              kind="ExternalInput")
    # packed wo: [100, m(12) * 1200] bf16
    tn["wo"] = nc.dram_tensor("wo", [D, NM * HID], BF16, kind="ExternalInput")
    tn["biasT"] = nc.dram_tensor("biasT", [128, HL * TT], F32,
                                 kind="ExternalInput")
    tn["y"] = nc.dram_tensor("y", [ROWS, HID], F32, kind="ExternalOutput")

    with tile.TileContext(nc) as tc:
        for _ in range(n_iters):
            _emit_iter(nc, tc, tn, scale)
    nc.compile()
    return nc


def _emit_iter(nc, tc, tn, scale):
    xT, wqk, wv, wo, biasT, y = (tn["xT"], tn["wqk"], tn["wv"], tn["wo"],
                                 tn["biasT"], tn["y"])

    with ExitStack() as ctx:
        pa = ctx.enter_context(tc.tile_pool(name="pa", bufs=1))
        vp = pa.tile([128, TT * VW], BF16, name="vp")
        ot = pa.tile([D, HL * S], BF16, name="ot")
        bias_sb = pa.tile([128, HL * TT], F32, name="bias_sb")
        ones1 = pa.tile([1, D], F32R, name="ones1")
        # wv in f32r: f32r matmuls with moving dim >=256 stream 2 cols/cycle
        # on HW, so V' at f32r halves its PE time AND needs no x bf16 copies.
        # Own pool: closes after V', freeing the space for attention-phase
        # tiles.
        pwv = ctx.enter_context(tc.tile_pool(name="pwv", bufs=1))
        wv_sb = pwv.tile([CK, CCH * HL * D], F32R, name="wv_sb")

        pqk = ctx.enter_context(tc.tile_pool(name="pqk", bufs=2))
        pwqk = ctx.enter_context(tc.tile_pool(name="pwqk", bufs=2))
        ppt = ctx.enter_context(tc.tile_pool(name="ppt", bufs=2))
        pnr = ctx.enter_context(tc.tile_pool(name="pnr", bufs=2))

        qk_tiles = {}
        w_tiles = {}

        def emit_wqk_dma(h):
            w_sb = pwqk.tile([CK, CCH * 2 * D], F32R, tag="wqk", name="w_sb")
            nc.sync.dma_start(out=w_sb, in_=wqk.ap()[h])
            _absorb(nc, w_sb)
            w_tiles[h] = w_sb

        def emit_proj(h):
            """Q/K projection of head h from resident x^T into rotating
            [D, S] f32r tiles. Uses the 2-slot transient PSUM pool."""
            w_sb = w_tiles.pop(h)
            qt = pqk.tile([D, S], F32R, tag="qt", name="qt")
            kt = pqk.tile([D, S], F32R, tag="kt", name="kt")
            for sb in range(4):
                for qk, dest in ((0, qt), (1, kt)):
                    acc = ptr.tile([D, 512], F32, tag="tr", name="acc")
                    for c in range(CCH):
                        _mm(nc, acc[:, :],
                            w_sb[:, c * 2 * D + qk * D: c * 2 * D + (qk + 1) * D],
                            xt[:, c * S + sb * 512: c * S + (sb + 1) * 512],
                            start=(c == 0), stop=(c == CCH - 1))
                    nc.vector.tensor_copy(
                        out=dest[:, sb * 512:(sb + 1) * 512], in_=acc[:, :])
            qk_tiles[h] = (qt, kt)

        def emit_vprime(tts):
            """V' for t-tiles `tts` (<=3 at a time), both jh head-halves,
            all f32r. accs: 2*len(tts) banks + 2 transient <= 8."""
            with tc.tile_pool(name=f"psv{tts[0]}", bufs=2 * len(tts),
                              space="PSUM") as psv:
                accs = {(jh, t): psv.tile([128, 3 * D], F32, tag="vacc",
                                          name="vacc")
                        for jh in range(2) for t in tts}
                for c in range(CCH):
                    for jh in range(2):
                        for t in tts:
                            _mm(nc, accs[(jh, t)][:, :],
                                xt[:, c * S + t * 128: c * S + t * 128 + 128],
                                wv_sb[:, c * HL * D + jh * 3 * D:
                                      c * HL * D + (jh + 1) * 3 * D],
                                start=(c == 0), stop=(c == CCH - 1))
                for (jh, t), acc in accs.items():
                    dst = vp[:, t * VW + jh * 3 * (D + 1):
                             t * VW + (jh + 1) * 3 * (D + 1)]
                    dst3 = dst.rearrange("p (h e) -> p h e", e=D + 1)
                    nc.any.tensor_copy(out=dst3[:, :, 0:D],
                                       in_=acc.rearrange(
                                           "p (h d) -> p h d", d=D))

        pending_norm = []

        def emit_norm_finish():
            """recip + rank-1 broadcast + in-place multiply for a previously
            evacuated (h, sh) block pair. Deferred one s-half window so the
            pb matmuls sit behind the next tt-loop in the static PE order
            (they depend on the slow DVE chain)."""
            if not pending_norm:
                return
            for col, srow in pending_norm.pop(0):
                srow1 = pnr.tile([1, 512], F32R, tag="srow1", name="srow1",
                                bufs=1)
                nc.sync.dma_start(out=srow1, in_=srow[4:5, :])
                rrow = pnr.tile([1, 512], F32R, tag="rrow", name="rrow",
                                bufs=1)
                with nc.allow_low_precision(reason="f32r recip of sums"):
                    nc.vector.reciprocal(out=rrow, in_=srow1)
                pb = ptr.tile([D, 512], F32, tag="tr", name="pb")
                _mm(nc, pb[:, :], ones1[0:1, :], rrow[0:1, :],
                    start=True, stop=True)
                nc.vector.tensor_mul(ot[:, col:col + 512],
                                     ot[:, col:col + 512], pb[:, :])

        def emit_attention_sh(h, sh, qt, kt):
            s0 = sh * 1024
            pos = [ppo.tile([D + 1, 512], F32, tag="po", name="po")
                   for _ in range(2)]
            for tt in range(TT):
                ss = pss.tile([128, 1024], F32, tag="ss", name="ss")
                for sbb in range(2):
                    _mm(nc, ss[:, sbb * 512:(sbb + 1) * 512],
                        kt[:, tt * 128:(tt + 1) * 128],
                        qt[:, s0 + sbb * 512: s0 + (sbb + 1) * 512],
                        start=True, stop=True)
                pt = ppt.tile([128, 1024], BF16, tag="pt", name="pt")
                nc.scalar.activation(
                    out=pt, in_=ss[:, :], func=EXP,
                    bias=bias_sb[:, h * TT + tt: h * TT + tt + 1],
                    scale=scale)
                for sbb in range(2):
                    _mm(nc, pos[sbb][:, :],
                        vp[:, tt * VW + h * (D + 1):
                           tt * VW + (h + 1) * (D + 1)],
                        pt[:, sbb * 512:(sbb + 1) * 512],
                        start=(tt == 0), stop=(tt == TT - 1))
            # evacuation: sums row + raw O^T copy free each po bank promptly
            grp = []
            for sbb in range(2):
                po = pos[sbb]
                col = h * S + s0 + sbb * 512
                srow = pnr.tile([5, 512], F32R, tag="srow", name="srow")
                nc.vector.tensor_copy(out=srow, in_=po[96:D + 1, :])
                nc.vector.tensor_copy(out=ot[:, col:col + 512],
                                      in_=po[0:D, :])
                grp.append((col, srow))
            pending_norm.append(grp)

        # ================= emission (priority) order =================
        nc.sync.dma_start(out=bias_sb, in_=biasT.ap())
        emit_wqk_dma(0)
        wv_sb3 = wv_sb.rearrange("p (c w) -> p c w", w=HL * D)
        wv_dr3 = wv.ap().rearrange("p (c w) -> p c w", w=HL * D)
        nc.sync.dma_start(out=wv_sb3[:, :, 0:3 * D], in_=wv_dr3[:, :, 0:3 * D])
        emit_wqk_dma(1)
        nc.sync.dma_start(out=wv_sb3[:, :, 3 * D:HL * D],
                          in_=wv_dr3[:, :, 3 * D:HL * D])
        nc.vector.memset(ones1.bitcast(F32), 1.0)
        nc.vector.tensor_copy(out=ones1, in_=ones1.bitcast(F32))
        # ones cols pre-set; V cols overwritten by the V' copies below.
        nc.vector.memset(vp, 1.0)

        with tc.tile_pool(name="pxt", bufs=1) as pxt:
            xt = pxt.tile([CK, CCH * S], F32R, name="xt")
            for c2 in range(2 * CCH):
                nc.sync.dma_start(
                    out=xt[:, c2 * 1024:(c2 + 1) * 1024],
                    in_=xT.ap()[(c2 // 2) * CK:(c2 // 2 + 1) * CK,
                                (c2 % 2) * 1024:((c2 % 2) + 1) * 1024])
                _absorb(nc, xt[:, c2 * 1024:(c2 + 1) * 1024])

            # shared 2-bank transient PSUM: proj accs and norm pb rotate
            with tc.tile_pool(name="ptr", bufs=2, space="PSUM") as ptr:
                # consume x chunks as they arrive: V' + head-0/1 proj
                emit_vprime(range(0, 3))
                emit_proj(0)
                emit_vprime(range(3, 6))
                emit_proj(1)
                emit_vprime(range(6, 9))
                emit_vprime(range(9, 12))
                emit_vprime(range(12, 15))
                emit_vprime(range(15, 16))

                with tc.tile_pool(name="pss", bufs=2, space="PSUM") as pss, \
                     tc.tile_pool(name="ppo", bufs=2, space="PSUM") as ppo:
                    for h in range(6):
                        if h >= 2:
                            emit_wqk_dma(h)
                            emit_proj(h)
                        qt, kt = qk_tiles.pop(h)
                        for sh in range(2):
                            emit_attention_sh(h, sh, qt, kt)
                            if len(pending_norm) > 1:
                                emit_norm_finish()
                    emit_norm_finish()
                    emit_norm_finish()

        # ============ output projection (wo resident in SBUF) ============
        pwo = ctx.enter_context(tc.tile_pool(name="pwo", bufs=1))
        pyb = ctx.enter_context(tc.tile_pool(name="pyb", bufs=4))
        wo_sb = pwo.tile([D, NM * HID], BF16, name="wo_sb")
        nc.sync.dma_start(out=wo_sb, in_=wo.ap())
        _absorb(nc, wo_sb)
        ot_r = ot.rearrange("p (r m) -> p r m", m=NM)
        with tc.tile_pool(name="psy", bufs=8, space="PSUM") as psy:
            for jb in range(3):
                pys = [psy.tile([128, 400], F32, tag="py", name="py")
                       for _ in range(8)]
                for m in range(NM):
                    for rt in range(8):
                        _mm(nc, pys[rt][:, :],
                            ot_r[:, rt * 128:(rt + 1) * 128, m],
                            wo_sb[:, m * HID + jb * 400:
                                  m * HID + (jb + 1) * 400],
                            start=(m == 0), stop=(m == NM - 1))
                for rt in range(8):
                    ysb = pyb.tile([128, 400], F32, tag="ysb", name="ysb")
                    nc.any.tensor_copy(out=ysb, in_=pys[rt][:, :])
                    nc.sync.dma_start(
                        out=y.ap()[rt * 128:(rt + 1) * 128,
                                   jb * 400:(jb + 1) * 400],
                        in_=ysb)


def make_core_inputs(x, alibi, attention_mask, wq, wk, wv, wo, layer_index):
    li = int(np.asarray(layer_index))
    inv = np.float32(1.0 / (li + 1))
    f32 = np.float32
    bf16 = ml_dtypes.bfloat16

    xTs = [np.ascontiguousarray(np.asarray(x[b], dtype=f32).T)
           for b in range(B)]

    # packed wo: wo_pk[d, m*1200 + n] = wo.T[m*100+d, n]
    woT = np.asarray(wo, dtype=f32).T                       # [1200, 1200]
    wo_pk = np.ascontiguousarray(
        woT.reshape(NM, D, HID).transpose(1, 0, 2).reshape(D, NM * HID)
    ).astype(bf16)

    per_group = []
    for g in range(HG):
        sl = slice(g * HL * D, (g + 1) * HL * D)
        # wqk[h, p, c*200 + qk*100 + d] = w{q,k}[g*600 + h*100 + d, c*120 + p]
        wq_g = np.asarray(wq, dtype=f32)[sl, :]             # [600, 1200]
        wk_g = np.asarray(wk, dtype=f32)[sl, :]
        wqk_pk = np.empty((HL, CK, CCH * 2 * D), dtype=f32)
        for h in range(HL):
            qh = wq_g[h * D:(h + 1) * D, :].T               # [1200, 100]
            kh = wk_g[h * D:(h + 1) * D, :].T
            both = np.concatenate(
                [qh.reshape(CCH, CK, D), kh.reshape(CCH, CK, D)],
                axis=2)                                     # [10, 120, 200]
            wqk_pk[h] = both.transpose(1, 0, 2).reshape(CK, CCH * 2 * D)
        # wv_pk[p, c*600 + col] = wv[g*600 + col, c*120 + p]
        wv_g = np.asarray(wv, dtype=f32)[sl, :].T           # [1200, 600]
        wv_pk = np.ascontiguousarray(
            wv_g.reshape(CCH, CK, HL * D).transpose(1, 0, 2)
            .reshape(CK, CCH * HL * D))
        per_group.append((np.ascontiguousarray(wqk_pk), wv_pk))

    in_maps = []
    for b in range(B):
        for g in range(HG):
            a = np.asarray(alibi, dtype=f32)[
                b * H + g * HL: b * H + (g + 1) * HL, 0, :]      # (6, S)
            msk = np.asarray(attention_mask, dtype=f32)[b, 0, 0, :S]
            bias = a * inv + msk[None, :]                        # (6, S)
            biasT = np.ascontiguousarray(
                bias.reshape(HL, TT, 128).transpose(2, 0, 1)
                .reshape(128, HL * TT))
            wqk_pk, wv_pk = per_group[g]
            in_maps.append({
                "xT": xTs[b], "wqk": wqk_pk, "wv": wv_pk,
                "wo": wo_pk, "biasT": biasT,
            })
    scale = float(np.float32(np.sqrt(np.float32(D))) * inv)
    return in_maps, scale


def run(trace=False, **inputs):
    in_maps, scale = make_core_inputs(**inputs)
    nc = build_program(scale)
    res = run_bass_kernel_spmd(nc, in_maps, core_ids=list(range(B * HG)),
                               trace=trace)
    out = np.empty((B, S, HID), dtype=np.float32)
    for b in range(B):
        for g in range(HG):
            out[b, g * ROWS:(g + 1) * ROWS, :] = res.results[b * HG + g]["y"]
    return out, res


def kernel(**inputs) -> np.ndarray:
    out, _ = run(trace=False, **inputs)
    return out
